# revision 50
# baseline (speedup 1.0000x reference)
"""Distributed KNN retrieval kernel for Trainium2 (8 NeuronCores).

Reference computation (B=64 queries, N=500k keys/values, D=256, k=32):
    q_proj = query @ Wq.T + bq
    scores = q_proj @ keys.T / sqrt(D)
    weights = softmax(scores)
    top_w, top_idx = top_k(weights, k)
    out = (values[top_idx], top_w)

Sharding: keys are split across 8 cores on the N axis (62500 each), and
each core's shard is further split into two halves of 31250 stacked on the
partition axis (rows 0-63 = batch for half 0, rows 64-127 = batch for
half 1) so every DVE/ACT op runs with all 128 partitions busy.

Two-stage retrieval: the device scans all keys at fp8-e4m3 precision
(selection only — score noise ~0.02 vs >=0.1 candidate margins), and the
host exactly rescores the top ~192 candidates per row in fp32/fp64 before
the final top-k, so the returned indices match the fp32 reference exactly.
fp8 keys halve HBM traffic twice over (16 MB per core instead of 64 MB).

Per core the device kernel:
  1. projects the queries (fp32 matmuls, bias + 1/sqrt(D) folded in,
     result cast to bf16 for the scoring matmuls),
  2. streams the group-interleaved packed fp8 key shard from HBM (one
     1 MB DMA delivers everything one 1024-column group needs),
  3. computes score chunks with bf16 x fp8 matmuls into PSUM (the two
     halves write disjoint 64-partition column groups of the PE array,
     which run concurrently),
  4. copies PSUM->SBUF through ScalarE with func=Exp, accumulating the
     per-row sum of exp(s) (softmax partials) as a side output,
  5. runs DVE max8/max_index per 1024-column segment to collect the top-8
     exp-space candidates (+ in-segment positions) of every segment,
  6. streams the candidate pools out incrementally.

The host merges the per-core candidate pools (~4200 candidates per batch
row), exactly rescores the top-192, picks the global top-k with
jax.lax.top_k tie-breaking, normalizes by the allreduced sum of exp, and
gathers the winning value rows. The segment top-8 pools contain the true
global top-32 unless one 1024-key segment holds >=9 of a row's global
top-32 (probability ~2e-5 for gaussian scores).
"""

import numpy as np

N_CORES = 8
B = 64
N_KEYS = 500000
D = 256
PER_CORE = N_KEYS // N_CORES  # 62500
HALF = PER_CORE // 2  # 31250

CHUNK = 512  # matmul free dim (PSUM bank limit for fp32 accum)
SEG = 1024  # top-8 segment size for DVE max8/max_index
GROUP = 1024  # cols per PSUM batch / exp-copy
RESCORE = 192  # candidates per row exactly rescored on the host


def _chunks(total, step):
    out = []
    o = 0
    while o < total:
        c = min(step, total - o)
        out.append((o, c))
        o += c
    return out


def _group_schedule(half):
    """Group column-ranges for the streaming loop: a few small groups up
    front so the ACT/DVE pipeline fills early, then full GROUP strides."""
    lead = [512, 512, 1024]
    out = []
    o = 0
    for c in lead:
        if o + c > half:
            break
        out.append((o, c))
        o += c
    for o2, c2 in _chunks(half - o, GROUP):
        out.append((o + o2, c2))
    return out


def _segment_schedule(half):
    """(offset, cols) of every max8 segment, in device emission order."""
    segs = []
    for g0, gcols in _group_schedule(half):
        for s0, scols in _chunks(gcols, SEG):
            segs.append((g0 + s0, scols))
    return segs


# ---------------------------------------------------------------------------
# sync-wait splitting: the pinned walrus build rejects >1 sem-wait per
# instruction ("Too many sync wait commands"). Moving excess waits onto
# same-engine NoOps inserted immediately before the instruction is
# semantically identical.
# ---------------------------------------------------------------------------


def _split_sync_waits(nc, max_waits=1):
    import concourse.mybir as mybir

    for fn in nc.m.functions:
        for bb in fn.blocks:
            insts = list(bb.instructions)
            new_insts = []
            changed = False
            for ins in insts:
                si = ins.sync_info
                waits = list(si.on_wait) if si is not None and si.on_wait else []
                if len(waits) > max_waits:
                    extra = waits[:-max_waits]
                    keep = waits[-max_waits:]
                    for ci, i0 in enumerate(range(0, len(extra), max_waits)):
                        chunk = extra[i0 : i0 + max_waits]
                        nop = mybir.InstNoOp(
                            name=f"{ins.name}-wsplit{ci}", ins=[], outs=[]
                        )
                        nop.engine = ins.engine
                        nop.sync_info = mybir.SyncInfo(on_wait=chunk, on_update=[])
                        new_insts.append(nop)
                    ins.sync_info = mybir.SyncInfo(
                        on_wait=keep,
                        on_update=list(si.on_update) if si.on_update else [],
                    )
                    changed = True
                new_insts.append(ins)
            if changed:
                try:
                    bb.instructions = new_insts
                except Exception:
                    bb.instructions.clear()
                    for i in new_insts:
                        bb.instructions.append(i)


# ---------------------------------------------------------------------------
# device kernel builder
# ---------------------------------------------------------------------------


def build_kernel(per_core=PER_CORE, b=B, d=D, split_waits=True, passes=1):
    """Build the per-core Bass module. Parameterized so CoreSim tests can
    run a scaled-down shard. passes>1 repeats the whole streaming loop
    (same inputs/outputs) — used only to measure steady-state device time
    by wall-clock deltas."""
    import concourse.bass as bass
    import concourse.mybir as mybir
    import concourse.tile as tile
    from concourse.tile import add_dep_helper

    assert per_core % 2 == 0
    half = per_core // 2
    n_seg = len(_segment_schedule(half))
    pool_w = n_seg * 8
    n_group = len(_group_schedule(half))
    kblocks = d // 128

    f32 = mybir.dt.float32
    bf16 = mybir.dt.bfloat16
    f8 = mybir.dt.float8e4
    u32 = mybir.dt.uint32

    nc = bass.Bass()
    # keysP: group-interleaved packed keys (fp8 e4m3 — selection precision
    # only; the host rescores the winners in fp32). For each column group g
    # the four streams (h0k0, h0k1, h1k0, h1k1) are stored back to back, so
    # a single DMA delivers everything group g's matmuls need.
    keysP = nc.dram_tensor("keysP", [128, 2 * per_core], f8, kind="ExternalInput")
    wqT = nc.dram_tensor("wqT", [d, d], f32, kind="ExternalInput")
    qT = nc.dram_tensor("qT", [d, b], f32, kind="ExternalInput")
    bq = nc.dram_tensor("bq", [d], f32, kind="ExternalInput")
    out_vals = nc.dram_tensor("out_vals", [2 * b, pool_w], f32, kind="ExternalOutput")
    out_idx = nc.dram_tensor("out_idx", [2 * b, pool_w], u32, kind="ExternalOutput")
    out_partials = nc.dram_tensor(
        "out_partials", [2 * b, n_group], f32, kind="ExternalOutput"
    )

    inv_sqrt_d = 1.0 / float(np.sqrt(d))
    bq2d = bq.rearrange("(a one) -> a one", one=1)

    with tile.TileContext(nc) as tc:
        with tc.tile_pool(name="persist", bufs=1) as persist:
            # --- query projection: q_projT[do, b] = (Wq @ query.T + bq)/sqrt(d)
            qpT = []  # lhsT tiles for the scores matmul, [128, b] per k-block
            with (
                tc.tile_pool(name="setup", bufs=1) as setup,
                tc.tile_pool(name="setup_ps", bufs=1, space="PSUM") as setup_ps,
            ):
                wq_sb = []
                qt_sb = []
                bq_sb = []
                for kb in range(kblocks):
                    wq_t = setup.tile([128, d], f32, name=f"wq_sb{kb}")
                    nc.sync.dma_start(wq_t[:, :], wqT[kb * 128 : (kb + 1) * 128, :])
                    wq_sb.append(wq_t)
                    qt_t = setup.tile([128, b], f32, name=f"qt_sb{kb}")
                    nc.sync.dma_start(qt_t[:, :], qT[kb * 128 : (kb + 1) * 128, :])
                    qt_sb.append(qt_t)
                    bq_t = setup.tile([128, 1], f32, name=f"bq_sb{kb}")
                    nc.sync.dma_start(bq_t[:, :], bq2d[kb * 128 : (kb + 1) * 128, :])
                    bq_sb.append(bq_t)
                for do in range(kblocks):
                    qp_ps = setup_ps.tile([128, b], f32, name=f"qp_ps{do}")
                    for kb in range(kblocks):
                        nc.tensor.matmul(
                            qp_ps[:, :],
                            wq_sb[kb][:, do * 128 : (do + 1) * 128],
                            qt_sb[kb][:, :],
                            start=(kb == 0),
                            stop=(kb == kblocks - 1),
                        )
                    qp_t = persist.tile([128, b], bf16, name=f"qpT{do}")
                    # (matmul + bq) * (1/sqrt(d)), cast to bf16 for scoring
                    nc.vector.tensor_scalar(
                        qp_t[:, :],
                        qp_ps[:, :],
                        bq_sb[do][:, :],
                        inv_sqrt_d,
                        op0=mybir.AluOpType.add,
                        op1=mybir.AluOpType.mult,
                    )
                    qpT.append(qp_t)

            pool_vals = persist.tile([128, pool_w], f32)
            pool_idx = persist.tile([128, pool_w], u32)
            partials = persist.tile([128, n_group], f32)
            # bias for the Exp activations; a memset tile (not a float) so no
            # const-AP DMA lands behind the 2 MB key loads on the HWDGE ring
            bias0 = persist.tile([128, 1], f32)
            nc.vector.memset(bias0[:, :], 0.0)

            # --- main streaming loop: one packed 2 MB DMA per group
            n_streams = 2 * kblocks
            with (
                tc.tile_pool(name="keys", bufs=8) as keys_pool,
                tc.tile_pool(name="win", bufs=8) as win_pool,
                tc.tile_pool(name="ps", bufs=4, space="PSUM") as ps_pool,
            ):
              for _pass in range(passes):
                seg_i = 0
                group_i = 0
                pack_off = 0
                prev_mm = None
                for g0, gcols in _group_schedule(half):
                    kt = keys_pool.tile([128, n_streams * GROUP], f8, name="kt")
                    nc.sync.dma_start(
                        kt[:, : n_streams * gcols],
                        keysP[:, pack_off : pack_off + n_streams * gcols],
                    )
                    pack_off += n_streams * gcols
                    ps = ps_pool.tile([128, GROUP], f32, name="ps")
                    first_mm = None
                    for c0, ccols in _chunks(gcols, CHUNK):
                        for h in range(2):
                            for kb in range(kblocks):
                                s = h * kblocks + kb
                                mm = nc.tensor.matmul(
                                    ps[h * b : (h + 1) * b, c0 : c0 + ccols],
                                    qpT[kb][:, :],
                                    kt[:, s * gcols + c0 : s * gcols + c0 + ccols],
                                    start=(kb == 0),
                                    stop=(kb == kblocks - 1),
                                    tile_position=(0, h * b),
                                )
                                if first_mm is None:
                                    first_mm = mm
                    # keep PE group-ordered so each Exp's PE-sem threshold is
                    # satisfied as soon as its own group's matmuls finish
                    if prev_mm is not None:
                        add_dep_helper(
                            first_mm.ins, prev_mm.ins, sync=False,
                            reason="PE group order",
                        )
                    prev_mm = mm
                    win = win_pool.tile([128, GROUP], f32, name="win")
                    nc.scalar.activation(
                        win[:, :gcols],
                        ps[:, :gcols],
                        mybir.ActivationFunctionType.Exp,
                        bias=bias0[:, :],
                        scale=1.0,
                        accum_out=partials[:, group_i : group_i + 1],
                    )
                    group_i += 1
                    for s0, scols in _chunks(gcols, SEG):
                        nc.vector.max(
                            pool_vals[:, seg_i * 8 : (seg_i + 1) * 8],
                            win[:, s0 : s0 + scols],
                        )
                        nc.vector.max_index(
                            pool_idx[:, seg_i * 8 : (seg_i + 1) * 8],
                            pool_vals[:, seg_i * 8 : (seg_i + 1) * 8],
                            win[:, s0 : s0 + scols],
                        )
                        seg_i += 1
                    # stream finished pool columns out so the final barrier
                    # only waits on the last slice
                    if _pass == passes - 1 and (
                        group_i == n_group or group_i % 8 == 0
                    ):
                        lo = 8 * (out_done := getattr(nc, "_out_done", 0))
                        hi = seg_i * 8
                        if hi > lo:
                            nc.sync.dma_start(
                                out_vals[:, lo:hi], pool_vals[:, lo:hi]
                            )
                            nc.sync.dma_start(
                                out_idx[:, lo:hi], pool_idx[:, lo:hi]
                            )
                            nc._out_done = seg_i
                assert seg_i == n_seg and group_i == n_group

            nc.sync.dma_start(out_partials[:, :], partials[:, :])

    if split_waits:
        _split_sync_waits(nc, max_waits=1)
    return nc


_CACHED = {}

# test/profiling hooks: kernel() stores the BassKernelResults of its last
# device run here; set PROFILE=True to request an NTFF trace.
PROFILE = False
TRACE_KWARGS = {}
LAST_RESULTS = None


def _get_kernel(per_core, b, d):
    key = (per_core, b, d)
    if key not in _CACHED:
        _CACHED[key] = build_kernel(per_core, b, d)
    return _CACHED[key]


# ---------------------------------------------------------------------------
# host wrapper
# ---------------------------------------------------------------------------


def _pack_keys(shard, half, d):
    """shard [per_core, d] fp32 -> group-interleaved packed bf16 [128, 2*per_core].

    Stream s = h*kblocks+kb of group g holds keysT[kb*128:(kb+1)*128,
    h*half + g0 : g0+gcols] where keysT = shard.T.
    """
    import ml_dtypes

    kblocks = d // 128
    shT = np.ascontiguousarray(shard.T).astype(ml_dtypes.float8_e4m3)  # [d, per_core]
    blocks = []
    for g0, gcols in _group_schedule(half):
        for h in range(2):
            for kb in range(kblocks):
                blocks.append(
                    shT[kb * 128 : (kb + 1) * 128, h * half + g0 : h * half + g0 + gcols]
                )
    return np.ascontiguousarray(np.concatenate(blocks, axis=1))


def _segment_offsets(half):
    return np.asarray([o for o, _c in _segment_schedule(half)], np.int64)


def kernel(query, keys, values, Wq, bq, k):
    from concourse.bass_utils import run_bass_kernel_spmd

    query = np.asarray(query, np.float32)
    keys = np.asarray(keys, np.float32)
    values = np.asarray(values, np.float32)
    Wq = np.asarray(Wq, np.float32)
    bq = np.asarray(bq, np.float32)
    k = int(k)

    b, d = query.shape
    n_keys = keys.shape[0]
    assert (b, d, n_keys) == (B, D, N_KEYS), (b, d, n_keys)
    per_core = n_keys // N_CORES
    half = per_core // 2

    nc = _get_kernel(per_core, b, d)

    wqT = np.ascontiguousarray(Wq.T)
    qT = np.ascontiguousarray(query.T)
    in_maps = []
    for c in range(N_CORES):
        shard = keys[c * per_core : (c + 1) * per_core]
        in_maps.append(
            {
                "keysP": _pack_keys(shard, half, d),
                "wqT": wqT,
                "qT": qT,
                "bq": bq,
            }
        )

    global LAST_RESULTS
    import time as _time

    res = None
    for attempt in range(3):
        try:
            res = run_bass_kernel_spmd(
                nc,
                in_maps,
                core_ids=list(range(N_CORES)),
                trace=PROFILE,
                **TRACE_KWARGS,
            )
            break
        except Exception:
            # the axon terminal occasionally reports the accelerator
            # unrecoverable on a first execution; a retry has always
            # succeeded in practice
            if attempt == 2:
                raise
            _time.sleep(3.0)
    LAST_RESULTS = res

    n_seg = len(_segment_schedule(half))
    pool_w = n_seg * 8
    seg_offs = _segment_offsets(half)  # [n_seg]

    # candidate global indices: core base + half base + segment offset + pos
    vals = np.stack([r["out_vals"] for r in res.results])  # [C, 128, pool_w]
    idxs = np.stack([r["out_idx"] for r in res.results]).astype(np.int64)
    parts = np.stack([r["out_partials"] for r in res.results])  # [C, 128, G]

    seg_base = np.repeat(seg_offs, 8)[None, None, :]  # [1,1,pool_w]
    core_base = (np.arange(N_CORES, dtype=np.int64) * per_core)[:, None, None]
    half_base = (np.arange(2, dtype=np.int64) * half)[None, :, None, None]
    gidx = idxs.reshape(N_CORES, 2, b, pool_w) + seg_base[None] + half_base
    gidx = gidx + core_base[:, None]  # broadcast -> [C, 2, b, pool_w]

    # reorder to per-batch-row candidate lists
    cand_v = (
        vals.reshape(N_CORES, 2, b, pool_w)
        .transpose(2, 0, 1, 3)
        .reshape(b, -1)
        .astype(np.float64)
    )
    cand_i = gidx.transpose(2, 0, 1, 3).reshape(b, -1)

    # allreduced softmax denominator (exp-space, bias 0). The device scores
    # are bf16-input matmuls (noise ~3e-3 per score); under the softmax
    # weighting those errors average out, so Z matches the fp32 Z to ~1e-5.
    z = parts.reshape(N_CORES, 2, b, -1).astype(np.float64).sum(axis=(0, 1, 3))  # [b]

    # device candidates were selected by approximate (fp8/bf16) score; pick
    # a generous top-RESCORE per row, rescore those exactly in fp32/fp64 on
    # the host (a few MFLOP), then take the true global top-k.
    t = min(max(RESCORE, 2 * k), cand_v.shape[1])
    part = np.argpartition(-cand_v, t - 1, axis=1)[:, :t]  # [b, t]
    cand_i_t = np.take_along_axis(cand_i, part, 1)  # [b, t]

    q_proj = (query @ Wq.T + bq).astype(np.float32)  # [b, d] like reference
    gk = keys[cand_i_t]  # [b, t, d]
    s_exact = np.einsum(
        "bd,btd->bt", q_proj.astype(np.float64), gk.astype(np.float64)
    ) / np.sqrt(d)
    w_exact = np.exp(s_exact).astype(np.float32)  # fp32 like reference weights

    # correct Z's head: replace the approximate exp of the rescored
    # candidates (where fp8 noise is concentrated in absolute terms) with
    # their exact values
    approx_head = np.take_along_axis(cand_v, part, 1).sum(axis=1)
    z = z - approx_head + np.exp(s_exact).sum(axis=1)

    top_idx = np.empty((b, k), np.int64)
    top_w = np.empty((b, k), np.float64)
    for i in range(b):
        order = np.lexsort((cand_i_t[i], -w_exact[i]))[:k]
        top_idx[i] = cand_i_t[i][order]
        top_w[i] = w_exact[i][order].astype(np.float64) / z[i]

    retrieved_values = values[top_idx]  # [b, k, d]
    retrieved_weights = top_w.astype(np.float32)
    return retrieved_values, retrieved_weights


# revision 51
# speedup vs baseline: 1.0672x; 1.0672x over previous
"""Distributed KNN retrieval kernel for Trainium2 (8 NeuronCores).

Reference computation (B=64 queries, N=500k keys/values, D=256, k=32):
    q_proj = query @ Wq.T + bq
    scores = q_proj @ keys.T / sqrt(D)
    weights = softmax(scores)
    top_w, top_idx = top_k(weights, k)
    out = (values[top_idx], top_w)

Sharding: keys are split across 8 cores on the N axis (62500 each), and
each core's shard is further split into two halves of 31250 stacked on the
partition axis (rows 0-63 = batch for half 0, rows 64-127 = batch for
half 1) so every DVE/ACT op runs with all 128 partitions busy.

Two-stage retrieval: the device scans all keys at fp8-e4m3 precision
(selection only — score noise ~0.02 vs >=0.1 candidate margins), and the
host exactly rescores the top ~192 candidates per row in fp32/fp64 before
the final top-k, so the returned indices match the fp32 reference exactly.
fp8 keys halve HBM traffic twice over (16 MB per core instead of 64 MB).

Per core the device kernel:
  1. projects the queries (fp32 matmuls, bias + 1/sqrt(D) folded in,
     result cast to bf16 for the scoring matmuls),
  2. streams the group-interleaved packed fp8 key shard from HBM (one
     1 MB DMA delivers everything one 1024-column group needs),
  3. computes score chunks with bf16 x fp8 matmuls into PSUM (the two
     halves write disjoint 64-partition column groups of the PE array,
     which run concurrently),
  4. copies PSUM->SBUF through ScalarE with func=Exp, accumulating the
     per-row sum of exp(s) (softmax partials) as a side output,
  5. runs DVE max8/max_index per 1024-column segment to collect the top-8
     exp-space candidates (+ in-segment positions) of every segment,
  6. streams the candidate pools out incrementally.

The host merges the per-core candidate pools (~4200 candidates per batch
row), exactly rescores the top-192, picks the global top-k with
jax.lax.top_k tie-breaking, normalizes by the allreduced sum of exp, and
gathers the winning value rows. The segment top-8 pools contain the true
global top-32 unless one 1024-key segment holds >=9 of a row's global
top-32 (probability ~2e-5 for gaussian scores).
"""

import numpy as np

N_CORES = 8
B = 64
N_KEYS = 500000
D = 256
PER_CORE = N_KEYS // N_CORES  # 62500
HALF = PER_CORE // 2  # 31250

CHUNK = 512  # matmul free dim (PSUM bank limit for fp32 accum)
SEG = 1024  # top-8 segment size for DVE max8/max_index
GROUP = 1024  # cols per PSUM batch / exp-copy
RESCORE = 192  # candidates per row exactly rescored on the host


def _chunks(total, step):
    out = []
    o = 0
    while o < total:
        c = min(step, total - o)
        out.append((o, c))
        o += c
    return out


def _group_schedule(half):
    """Group column-ranges for the streaming loop: a few small groups up
    front so the ACT/DVE pipeline fills early, then full GROUP strides."""
    lead = [256, 512, 1024]
    out = []
    o = 0
    for c in lead:
        if o + c > half:
            break
        out.append((o, c))
        o += c
    for o2, c2 in _chunks(half - o, GROUP):
        out.append((o + o2, c2))
    return out


def _segment_schedule(half):
    """(offset, cols) of every max8 segment, in device emission order."""
    segs = []
    for g0, gcols in _group_schedule(half):
        for s0, scols in _chunks(gcols, SEG):
            segs.append((g0 + s0, scols))
    return segs


# ---------------------------------------------------------------------------
# sync-wait splitting: the pinned walrus build rejects >1 sem-wait per
# instruction ("Too many sync wait commands"). Moving excess waits onto
# same-engine NoOps inserted immediately before the instruction is
# semantically identical.
# ---------------------------------------------------------------------------


def _split_sync_waits(nc, max_waits=1):
    import concourse.mybir as mybir

    for fn in nc.m.functions:
        for bb in fn.blocks:
            insts = list(bb.instructions)
            new_insts = []
            changed = False
            for ins in insts:
                si = ins.sync_info
                waits = list(si.on_wait) if si is not None and si.on_wait else []
                if len(waits) > max_waits:
                    extra = waits[:-max_waits]
                    keep = waits[-max_waits:]
                    for ci, i0 in enumerate(range(0, len(extra), max_waits)):
                        chunk = extra[i0 : i0 + max_waits]
                        nop = mybir.InstNoOp(
                            name=f"{ins.name}-wsplit{ci}", ins=[], outs=[]
                        )
                        nop.engine = ins.engine
                        nop.sync_info = mybir.SyncInfo(on_wait=chunk, on_update=[])
                        new_insts.append(nop)
                    ins.sync_info = mybir.SyncInfo(
                        on_wait=keep,
                        on_update=list(si.on_update) if si.on_update else [],
                    )
                    changed = True
                new_insts.append(ins)
            if changed:
                try:
                    bb.instructions = new_insts
                except Exception:
                    bb.instructions.clear()
                    for i in new_insts:
                        bb.instructions.append(i)


# ---------------------------------------------------------------------------
# device kernel builder
# ---------------------------------------------------------------------------


def build_kernel(per_core=PER_CORE, b=B, d=D, split_waits=True, passes=1):
    """Build the per-core Bass module. Parameterized so CoreSim tests can
    run a scaled-down shard. passes>1 repeats the whole streaming loop
    (same inputs/outputs) — used only to measure steady-state device time
    by wall-clock deltas."""
    import concourse.bass as bass
    import concourse.mybir as mybir
    import concourse.tile as tile
    from concourse.tile import add_dep_helper

    assert per_core % 2 == 0
    half = per_core // 2
    n_seg = len(_segment_schedule(half))
    pool_w = n_seg * 8
    n_group = len(_group_schedule(half))
    kblocks = d // 128

    f32 = mybir.dt.float32
    bf16 = mybir.dt.bfloat16
    f8 = mybir.dt.float8e4
    u32 = mybir.dt.uint32

    nc = bass.Bass()
    # keysP: group-interleaved packed keys (fp8 e4m3 — selection precision
    # only; the host rescores the winners in fp32). For each column group g
    # the four streams (h0k0, h0k1, h1k0, h1k1) are stored back to back, so
    # a single DMA delivers everything group g's matmuls need.
    keysP = nc.dram_tensor("keysP", [128, 2 * per_core], f8, kind="ExternalInput")
    wqT = nc.dram_tensor("wqT", [d, d], f32, kind="ExternalInput")
    qT = nc.dram_tensor("qT", [d, b], f32, kind="ExternalInput")
    bq = nc.dram_tensor("bq", [d], f32, kind="ExternalInput")
    out_vals = nc.dram_tensor("out_vals", [2 * b, pool_w], f32, kind="ExternalOutput")
    out_idx = nc.dram_tensor("out_idx", [2 * b, pool_w], u32, kind="ExternalOutput")
    out_partials = nc.dram_tensor(
        "out_partials", [2 * b, n_group], f32, kind="ExternalOutput"
    )

    inv_sqrt_d = 1.0 / float(np.sqrt(d))
    bq2d = bq.rearrange("(a one) -> a one", one=1)

    with tile.TileContext(nc) as tc:
        with tc.tile_pool(name="persist", bufs=1) as persist:
            # --- query projection: q_projT[do, b] = (Wq @ query.T + bq)/sqrt(d)
            qpT = []  # lhsT tiles for the scores matmul, [128, b] per k-block
            with (
                tc.tile_pool(name="setup", bufs=1) as setup,
                tc.tile_pool(name="setup_ps", bufs=1, space="PSUM") as setup_ps,
            ):
                wq_sb = []
                qt_sb = []
                bq_sb = []
                for kb in range(kblocks):
                    wq_t = setup.tile([128, d], f32, name=f"wq_sb{kb}")
                    nc.sync.dma_start(wq_t[:, :], wqT[kb * 128 : (kb + 1) * 128, :])
                    wq_sb.append(wq_t)
                    qt_t = setup.tile([128, b], f32, name=f"qt_sb{kb}")
                    nc.sync.dma_start(qt_t[:, :], qT[kb * 128 : (kb + 1) * 128, :])
                    qt_sb.append(qt_t)
                    bq_t = setup.tile([128, 1], f32, name=f"bq_sb{kb}")
                    nc.sync.dma_start(bq_t[:, :], bq2d[kb * 128 : (kb + 1) * 128, :])
                    bq_sb.append(bq_t)
                for do in range(kblocks):
                    qp_ps = setup_ps.tile([128, b], f32, name=f"qp_ps{do}")
                    for kb in range(kblocks):
                        nc.tensor.matmul(
                            qp_ps[:, :],
                            wq_sb[kb][:, do * 128 : (do + 1) * 128],
                            qt_sb[kb][:, :],
                            start=(kb == 0),
                            stop=(kb == kblocks - 1),
                        )
                    qp_t = persist.tile([128, b], bf16, name=f"qpT{do}")
                    # (matmul + bq) * (1/sqrt(d)), cast to bf16 for scoring
                    nc.vector.tensor_scalar(
                        qp_t[:, :],
                        qp_ps[:, :],
                        bq_sb[do][:, :],
                        inv_sqrt_d,
                        op0=mybir.AluOpType.add,
                        op1=mybir.AluOpType.mult,
                    )
                    qpT.append(qp_t)

            pool_vals = persist.tile([128, pool_w], f32)
            pool_idx = persist.tile([128, pool_w], u32)
            partials = persist.tile([128, n_group], f32)
            # bias for the Exp activations; a memset tile (not a float) so no
            # const-AP DMA lands behind the 2 MB key loads on the HWDGE ring
            bias0 = persist.tile([128, 1], f32)
            nc.vector.memset(bias0[:, :], 0.0)

            # --- main streaming loop: one packed 2 MB DMA per group
            n_streams = 2 * kblocks
            with (
                tc.tile_pool(name="keys", bufs=8) as keys_pool,
                tc.tile_pool(name="win", bufs=8) as win_pool,
                tc.tile_pool(name="ps", bufs=4, space="PSUM") as ps_pool,
            ):
              for _pass in range(passes):
                seg_i = 0
                group_i = 0
                pack_off = 0
                prev_mm = None
                for g0, gcols in _group_schedule(half):
                    kt = keys_pool.tile([128, n_streams * GROUP], f8, name="kt")
                    nc.sync.dma_start(
                        kt[:, : n_streams * gcols],
                        keysP[:, pack_off : pack_off + n_streams * gcols],
                    )
                    pack_off += n_streams * gcols
                    ps = ps_pool.tile([128, GROUP], f32, name="ps")
                    first_mm = None
                    for c0, ccols in _chunks(gcols, CHUNK):
                        for h in range(2):
                            for kb in range(kblocks):
                                s = h * kblocks + kb
                                mm = nc.tensor.matmul(
                                    ps[h * b : (h + 1) * b, c0 : c0 + ccols],
                                    qpT[kb][:, :],
                                    kt[:, s * gcols + c0 : s * gcols + c0 + ccols],
                                    start=(kb == 0),
                                    stop=(kb == kblocks - 1),
                                    tile_position=(0, h * b),
                                )
                                if first_mm is None:
                                    first_mm = mm
                    # keep PE group-ordered so each Exp's PE-sem threshold is
                    # satisfied as soon as its own group's matmuls finish
                    if prev_mm is not None:
                        add_dep_helper(
                            first_mm.ins, prev_mm.ins, sync=False,
                            reason="PE group order",
                        )
                    prev_mm = mm
                    win = win_pool.tile([128, GROUP], f32, name="win")
                    nc.scalar.activation(
                        win[:, :gcols],
                        ps[:, :gcols],
                        mybir.ActivationFunctionType.Exp,
                        bias=bias0[:, :],
                        scale=1.0,
                        accum_out=partials[:, group_i : group_i + 1],
                    )
                    group_i += 1
                    for s0, scols in _chunks(gcols, SEG):
                        nc.vector.max(
                            pool_vals[:, seg_i * 8 : (seg_i + 1) * 8],
                            win[:, s0 : s0 + scols],
                        )
                        nc.vector.max_index(
                            pool_idx[:, seg_i * 8 : (seg_i + 1) * 8],
                            pool_vals[:, seg_i * 8 : (seg_i + 1) * 8],
                            win[:, s0 : s0 + scols],
                        )
                        seg_i += 1
                    # stream finished pool columns out so the final barrier
                    # only waits on the last slice
                    if _pass == passes - 1 and (
                        group_i == n_group or group_i % 8 == 0
                    ):
                        lo = 8 * (out_done := getattr(nc, "_out_done", 0))
                        hi = seg_i * 8
                        if hi > lo:
                            nc.sync.dma_start(
                                out_vals[:, lo:hi], pool_vals[:, lo:hi]
                            )
                            nc.sync.dma_start(
                                out_idx[:, lo:hi], pool_idx[:, lo:hi]
                            )
                            nc._out_done = seg_i
                assert seg_i == n_seg and group_i == n_group

            nc.sync.dma_start(out_partials[:, :], partials[:, :])

    if split_waits:
        _split_sync_waits(nc, max_waits=1)
    return nc


_CACHED = {}

# test/profiling hooks: kernel() stores the BassKernelResults of its last
# device run here; set PROFILE=True to request an NTFF trace.
PROFILE = False
TRACE_KWARGS = {}
LAST_RESULTS = None


def _get_kernel(per_core, b, d):
    key = (per_core, b, d)
    if key not in _CACHED:
        _CACHED[key] = build_kernel(per_core, b, d)
    return _CACHED[key]


# ---------------------------------------------------------------------------
# host wrapper
# ---------------------------------------------------------------------------


def _pack_keys(shard, half, d):
    """shard [per_core, d] fp32 -> group-interleaved packed bf16 [128, 2*per_core].

    Stream s = h*kblocks+kb of group g holds keysT[kb*128:(kb+1)*128,
    h*half + g0 : g0+gcols] where keysT = shard.T.
    """
    import ml_dtypes

    kblocks = d // 128
    shT = np.ascontiguousarray(shard.T).astype(ml_dtypes.float8_e4m3)  # [d, per_core]
    blocks = []
    for g0, gcols in _group_schedule(half):
        for h in range(2):
            for kb in range(kblocks):
                blocks.append(
                    shT[kb * 128 : (kb + 1) * 128, h * half + g0 : h * half + g0 + gcols]
                )
    return np.ascontiguousarray(np.concatenate(blocks, axis=1))


def _segment_offsets(half):
    return np.asarray([o for o, _c in _segment_schedule(half)], np.int64)


def kernel(query, keys, values, Wq, bq, k):
    from concourse.bass_utils import run_bass_kernel_spmd

    query = np.asarray(query, np.float32)
    keys = np.asarray(keys, np.float32)
    values = np.asarray(values, np.float32)
    Wq = np.asarray(Wq, np.float32)
    bq = np.asarray(bq, np.float32)
    k = int(k)

    b, d = query.shape
    n_keys = keys.shape[0]
    assert (b, d, n_keys) == (B, D, N_KEYS), (b, d, n_keys)
    per_core = n_keys // N_CORES
    half = per_core // 2

    nc = _get_kernel(per_core, b, d)

    wqT = np.ascontiguousarray(Wq.T)
    qT = np.ascontiguousarray(query.T)
    in_maps = []
    for c in range(N_CORES):
        shard = keys[c * per_core : (c + 1) * per_core]
        in_maps.append(
            {
                "keysP": _pack_keys(shard, half, d),
                "wqT": wqT,
                "qT": qT,
                "bq": bq,
            }
        )

    global LAST_RESULTS
    import time as _time

    res = None
    for attempt in range(3):
        try:
            res = run_bass_kernel_spmd(
                nc,
                in_maps,
                core_ids=list(range(N_CORES)),
                trace=PROFILE,
                **TRACE_KWARGS,
            )
            break
        except Exception:
            # the axon terminal occasionally reports the accelerator
            # unrecoverable on a first execution; a retry has always
            # succeeded in practice
            if attempt == 2:
                raise
            _time.sleep(3.0)
    LAST_RESULTS = res

    n_seg = len(_segment_schedule(half))
    pool_w = n_seg * 8
    seg_offs = _segment_offsets(half)  # [n_seg]

    # candidate global indices: core base + half base + segment offset + pos
    vals = np.stack([r["out_vals"] for r in res.results])  # [C, 128, pool_w]
    idxs = np.stack([r["out_idx"] for r in res.results]).astype(np.int64)
    parts = np.stack([r["out_partials"] for r in res.results])  # [C, 128, G]

    seg_base = np.repeat(seg_offs, 8)[None, None, :]  # [1,1,pool_w]
    core_base = (np.arange(N_CORES, dtype=np.int64) * per_core)[:, None, None]
    half_base = (np.arange(2, dtype=np.int64) * half)[None, :, None, None]
    gidx = idxs.reshape(N_CORES, 2, b, pool_w) + seg_base[None] + half_base
    gidx = gidx + core_base[:, None]  # broadcast -> [C, 2, b, pool_w]

    # reorder to per-batch-row candidate lists
    cand_v = (
        vals.reshape(N_CORES, 2, b, pool_w)
        .transpose(2, 0, 1, 3)
        .reshape(b, -1)
        .astype(np.float64)
    )
    cand_i = gidx.transpose(2, 0, 1, 3).reshape(b, -1)

    # allreduced softmax denominator (exp-space, bias 0). The device scores
    # are bf16-input matmuls (noise ~3e-3 per score); under the softmax
    # weighting those errors average out, so Z matches the fp32 Z to ~1e-5.
    z = parts.reshape(N_CORES, 2, b, -1).astype(np.float64).sum(axis=(0, 1, 3))  # [b]

    # device candidates were selected by approximate (fp8/bf16) score; pick
    # a generous top-RESCORE per row, rescore those exactly in fp32/fp64 on
    # the host (a few MFLOP), then take the true global top-k.
    t = min(max(RESCORE, 2 * k), cand_v.shape[1])
    part = np.argpartition(-cand_v, t - 1, axis=1)[:, :t]  # [b, t]
    cand_i_t = np.take_along_axis(cand_i, part, 1)  # [b, t]

    q_proj = (query @ Wq.T + bq).astype(np.float32)  # [b, d] like reference
    gk = keys[cand_i_t]  # [b, t, d]
    s_exact = np.einsum(
        "bd,btd->bt", q_proj.astype(np.float64), gk.astype(np.float64)
    ) / np.sqrt(d)
    w_exact = np.exp(s_exact).astype(np.float32)  # fp32 like reference weights

    # correct Z's head: replace the approximate exp of the rescored
    # candidates (where fp8 noise is concentrated in absolute terms) with
    # their exact values
    approx_head = np.take_along_axis(cand_v, part, 1).sum(axis=1)
    z = z - approx_head + np.exp(s_exact).sum(axis=1)

    top_idx = np.empty((b, k), np.int64)
    top_w = np.empty((b, k), np.float64)
    for i in range(b):
        order = np.lexsort((cand_i_t[i], -w_exact[i]))[:k]
        top_idx[i] = cand_i_t[i][order]
        top_w[i] = w_exact[i][order].astype(np.float64) / z[i]

    retrieved_values = values[top_idx]  # [b, k, d]
    retrieved_weights = top_w.astype(np.float32)
    return retrieved_values, retrieved_weights
